# revision 21
# baseline (speedup 1.0000x reference)
"""Trainium2 Bass kernel for nn_MultiHeadCulturalAttention.

Sharding (8 cores, SPMD single program with a partition-id branch):
  cores 0-3: "regular" branch — (batch b = core//2), 3 heads of hd=128 each
  cores 4-7: "cultural" branch — (batch b = (core-4)//2), 1 head of hd=384

All streaming operands are float16 (PE runs 16-bit at full rate with fast
weight loads; DVE gets 2-4x modes); accumulation is fp32 in PSUM.
Softmax denominators come from DVE-accumulated exp tiles reduced by a
ones[128,128] broadcast-matmul per t-window, then a fast-approx DVE
reciprocal + multiply (no gpsimd, no DMA round-trips).

The regular branch's attention is exp-paced on the scalar engine, so all
remaining PE work is software-pipelined into its idle slots: only Q/K of
head-chunk 0 are projected up front; the V projection runs inside the
first attention window, Q/K of chunks 1-2 inside later windows, and the
folded output projection (wfold = branch_out_w @ out_w_half) inside the
last map. The cultural branch (PE-paced) projects Q/K densely, folds at
the end. Host sums 4 fp32 partials per batch plus a constant bias row.
"""
import numpy as np

import concourse.bass as bass
import concourse.mybir as mybir
from concourse import bacc
from concourse.tile import TileContext
from concourse.bass_utils import run_bass_kernel_spmd

F32 = mybir.dt.float32
F16 = mybir.dt.float16
AF = mybir.ActivationFunctionType
ALU = mybir.AluOpType

B, T, E = 2, 2048, 768
NE = E // 128            # 6 e-chunks
NT = T // 128            # 16 tiles along seq
F = 384                  # per-core projection width (3 reg heads / 1 cul head)
NF = F // 128            # 3 f-tiles
SCALE_REG = float(128 ** -0.5)
SCALE_CUL = float(384 ** -0.5)
GRP = 8                  # exp-tile accumulation group size

_NC_CACHE = None


def _build_nc():
    nc = bacc.Bacc()
    d_xT = [nc.declare_dram_parameter(f"xT{e}", [128, T], F16, isOutput=False)
            for e in range(NE)]
    d_wq = nc.declare_dram_parameter("wq", [128, NE * F], F16, isOutput=False)
    d_wk = nc.declare_dram_parameter("wk", [128, NE * F], F16, isOutput=False)
    d_wv = nc.declare_dram_parameter("wv", [128, NE * F], F16, isOutput=False)
    d_qb = nc.declare_dram_parameter("qb", [128, NF], F32, isOutput=False)
    d_kb = nc.declare_dram_parameter("kb", [128, NF], F32, isOutput=False)
    d_attn = nc.declare_dram_parameter("attn", [128, NT], F32, isOutput=False)
    d_wfold = nc.declare_dram_parameter("wfold", [128, NF * E], F16, isOutput=False)
    d_em = nc.declare_dram_parameter("em", [NT, 128, T], F16, isOutput=False)
    d_out = nc.declare_dram_parameter("out", [T, E], F32, isOutput=True)

    with TileContext(nc) as tc:
        pid = nc.partition_id()
        from contextlib import ExitStack
        with ExitStack() as stk:
            # ---- persistent pools ----
            p_small = stk.enter_context(tc.tile_pool(name="small", bufs=1))
            p_qt = stk.enter_context(tc.tile_pool(name="qt", bufs=1))
            p_kt = stk.enter_context(tc.tile_pool(name="kt", bufs=1))
            p_v = stk.enter_context(tc.tile_pool(name="vp", bufs=1))
            p_outT = stk.enter_context(tc.tile_pool(name="outT", bufs=1))
            p_wfold = stk.enter_context(tc.tile_pool(name="wfp", bufs=1))
            p_xw = stk.enter_context(tc.tile_pool(name="xw", bufs=1))

            sb_ones = p_small.tile([128, 512], F16)
            sb_qb = p_small.tile([128, NF], F32)
            sb_kb = p_small.tile([128, NF], F32)
            sb_attn = p_small.tile([128, NT], F32)
            sb_wfold = p_wfold.tile([128, NF * E], F16)
            nc.vector.memset(sb_ones[:], 1.0)
            nc.sync.dma_start(out=sb_qb[:], in_=d_qb[:])
            nc.sync.dma_start(out=sb_kb[:], in_=d_kb[:])
            nc.sync.dma_start(out=sb_attn[:], in_=d_attn[:])

            outT = [p_outT.tile([128, T], F16, tag=f"outT{j}", name=f"outT{j}")
                    for j in range(NF)]
            sb_q = [p_qt.tile([128, T], F16, tag=f"q{f}", name=f"qf{f}")
                    for f in range(NF)]
            sb_k = [p_kt.tile([128, T], F16, tag=f"k{f}", name=f"kf{f}")
                    for f in range(NF)]
            sb_v = [p_v.tile([128, F], F16, tag=f"v{s}", name=f"vs{s}")
                    for s in range(NT)]

            sb_wq = p_xw.tile([128, NE * F], F16, tag="wq")
            sb_wk = p_xw.tile([128, NE * F], F16, tag="wk")
            sb_wv = p_xw.tile([128, NE * F], F16, tag="wv")
            sb_x = p_xw.tile([128, NE * T], F16, tag="x")
            nc.sync.dma_start(out=sb_wq[:], in_=d_wq[:])
            nc.scalar.dma_start(out=sb_wk[:], in_=d_wk[:])
            nc.gpsimd.dma_start(out=sb_wv[:], in_=d_wv[:])
            dma_engs = (nc.sync, nc.scalar, nc.gpsimd)
            for tq in range(4):
                for e in range(NE):
                    dma_engs[(tq * NE + e) % 3].dma_start(
                        out=sb_x[:, e * T + tq * 512: e * T + (tq + 1) * 512],
                        in_=d_xT[e][:, tq * 512:(tq + 1) * 512])
            nc.sync.dma_start(out=sb_wfold[:], in_=d_wfold[:])

            # ---- projection chain helpers (emit into a given PSUM pool) ----
            def qk_chain(pool, which, f, tq):
                """One [128,512] projection chain for Q^T/K^T chunk f,
                T-quarter tq; DVE evacuation with per-partition bias."""
                sb_w = sb_wq if which == "q" else sb_wk
                sb_b = sb_qb if which == "q" else sb_kb
                dst = sb_q[f] if which == "q" else sb_k[f]
                c0 = tq * 512
                ps = pool.tile([128, 512], F32, tag="psc", name="ps_qk")
                for e in range(NE):
                    nc.tensor.matmul(
                        ps[:],
                        lhsT=sb_w[:, e * F + f * 128: e * F + (f + 1) * 128],
                        rhs=sb_x[:, e * T + c0: e * T + c0 + 512],
                        start=(e == 0), stop=(e == NE - 1))
                nc.vector.tensor_scalar_add(dst[:, c0:c0 + 512], ps[:],
                                            sb_b[:, f:f + 1])

            def v_chain(pool, s, engine="dve"):
                """V[s-block]: [128, F] = sum_e x^T[e, s]ᵀ wv[e]."""
                ps = pool.tile([128, F], F32, tag="psc", name="ps_vc")
                for e in range(NE):
                    nc.tensor.matmul(
                        ps[:], lhsT=sb_x[:, e * T + s * 128:(e * T) + (s + 1) * 128],
                        rhs=sb_wv[:, e * F:(e + 1) * F],
                        start=(e == 0), stop=(e == NE - 1))
                if engine == "dve":
                    nc.vector.tensor_copy(sb_v[s][:], ps[:])
                else:
                    nc.scalar.activation(sb_v[s][:], ps[:], AF.Copy,
                                         bias=0.0, scale=1.0)

            def fold_tt(p_pf, p_fin, tt, fin_engine):
                ps_f = p_pf.tile([128, E], F32, tag="pf", name="ps_f")
                for c in range(NF):
                    for e0, e1 in ((0, 512), (512, 768)):
                        nc.tensor.matmul(
                            ps_f[:, e0:e1],
                            lhsT=outT[c][:, tt * 128:(tt + 1) * 128],
                            rhs=sb_wfold[:, c * E + e0: c * E + e1],
                            start=(c == 0), stop=(c == NF - 1))
                fin = p_fin.tile([128, E], F32, tag="fin", name="fin")
                if fin_engine == "act":
                    nc.scalar.activation(fin[:], ps_f[:], AF.Copy,
                                         bias=0.0, scale=1.0)
                else:
                    nc.vector.tensor_copy(fin[:], ps_f[:])
                nc.sync.dma_start(out=d_out[tt * 128:(tt + 1) * 128, :], in_=fin[:])

            # ---- common: warmup + just enough Q0/K0/V for window (0,0) ----
            with tc.tile_pool(name="wu", bufs=2, space="PSUM") as p_wu:
                ps_wu = p_wu.tile([128, 512], F32, tag="wup", name="ps_wu")
                for _ in range(20):
                    nc.tensor.matmul(ps_wu[:], lhsT=sb_ones[:, 0:128],
                                     rhs=sb_ones[:], start=True, stop=True)
                qk_chain(p_wu, "q", 0, 0)
                qk_chain(p_wu, "k", 0, 0)
                v_chain(p_wu, 0)
                v_chain(p_wu, 1)
                qk_chain(p_wu, "q", 0, 1)

            n_grp = NT // GRP

            def attention(maps, scale, sb_em, inject, fold_last):
                """maps: (chunks, avs, t_win) per map; inject(pool, mi, ti, s)
                issues extra PE work into the exp-paced pipeline. fold_last:
                open fold pools inside the last map; window ti's fold runs
                deferred inside window ti+1's s-loop (the final window's
                fold trails)."""
                from contextlib import ExitStack as ES
                with ES() as astk:
                    p_wt = astk.enter_context(tc.tile_pool(name="wt", bufs=3))
                    p_acc = astk.enter_context(tc.tile_pool(name="acc", bufs=2))
                    p_rec = astk.enter_context(tc.tile_pool(name="rec", bufs=2))
                    for mi, (chunks, avs, t_win, po_bufs, inj_bufs,
                             psc_bufs) in enumerate(maps):
                        n_tq = t_win // 512
                        n_tt = t_win // 128
                        mstk = ES()
                        p_psc = mstk.enter_context(tc.tile_pool(
                            name=f"psc{mi}", bufs=psc_bufs, space="PSUM"))
                        p_po = mstk.enter_context(tc.tile_pool(
                            name=f"po{mi}", bufs=po_bufs, space="PSUM"))
                        p_inj = mstk.enter_context(tc.tile_pool(
                            name=f"pinj{mi}", bufs=inj_bufs, space="PSUM")) \
                            if inj_bufs else None
                        p_pf = p_fin = None
                        if fold_last and mi == len(maps) - 1:
                            pf_bufs = 2 if t_win == 512 and len(maps) > 1 else 1
                            p_pf = mstk.enter_context(tc.tile_pool(
                                name="pfr", bufs=pf_bufs, space="PSUM"))
                            p_fin = mstk.enter_context(
                                tc.tile_pool(name="finr", bufs=3))
                        fold_stride = NT // n_tt
                        for ti in range(T // t_win):
                            t0 = ti * t_win
                            ps_o = [p_po.tile([128, t_win], F32, tag=f"po{j}",
                                              name=f"ps_o{j}")
                                    for j in range(len(avs))]
                            accs = [p_acc.tile([128, t_win], F16, tag=f"acc{g}",
                                               name=f"acc{g}")
                                    for g in range(n_grp)]
                            for s in range(NT):
                                inject(p_inj if p_inj is not None else p_psc,
                                       mi, ti, s)
                                if (p_pf is not None and ti > 0
                                        and s % fold_stride == fold_stride - 1):
                                    tt = (ti - 1) * n_tt + s // fold_stride
                                    fold_tt(p_pf, p_fin, tt, "dve")
                                ps_sc = p_psc.tile([128, t_win], F32, tag="psc",
                                                   name="ps_sc")
                                for ci, c in enumerate(chunks):
                                    for tq in range(n_tq):
                                        nc.tensor.matmul(
                                            ps_sc[:, tq * 512:(tq + 1) * 512],
                                            lhsT=sb_k[c][:, s * 128:(s + 1) * 128],
                                            rhs=sb_q[c][:, t0 + tq * 512:
                                                        t0 + (tq + 1) * 512],
                                            start=(ci == 0),
                                            stop=(ci == len(chunks) - 1))
                                wt = p_wt.tile([128, t_win], F16, tag="wt",
                                               name="wt")
                                if sb_em is not None:
                                    wt0 = p_wt.tile([128, t_win], F16, tag="wt0",
                                                    name="wt0")
                                    nc.scalar.activation(wt0[:], ps_sc[:], AF.Exp,
                                                         bias=0.0, scale=scale)
                                    nc.vector.tensor_tensor(
                                        wt[:], wt0[:],
                                        sb_em[:, s * T + t0: s * T + t0 + t_win],
                                        ALU.mult)
                                else:
                                    nc.scalar.activation(
                                        wt[:], ps_sc[:], AF.Exp,
                                        bias=sb_attn[:, s:s + 1], scale=scale)
                                g = s // GRP
                                if s % GRP == 0:
                                    nc.vector.tensor_copy(accs[g][:], wt[:])
                                else:
                                    nc.vector.tensor_tensor(accs[g][:], accs[g][:],
                                                            wt[:], ALU.add)
                                for j, (vc, _oidx) in enumerate(avs):
                                    for tq in range(n_tq):
                                        nc.tensor.matmul(
                                            ps_o[j][:, tq * 512:(tq + 1) * 512],
                                            lhsT=sb_v[s][:, vc * 128:(vc + 1) * 128],
                                            rhs=wt[:, tq * 512:(tq + 1) * 512],
                                            start=(s == 0), stop=(s == NT - 1))
                            ps_den = p_psc.tile([128, t_win], F32, tag="psc",
                                                name="ps_den")
                            for g in range(n_grp):
                                for tq in range(n_tq):
                                    nc.tensor.matmul(
                                        ps_den[:, tq * 512:(tq + 1) * 512],
                                        lhsT=sb_ones[:, 0:128],
                                        rhs=accs[g][:, tq * 512:(tq + 1) * 512],
                                        start=(g == 0), stop=(g == n_grp - 1))
                            rec = p_rec.tile([128, t_win], F32, tag="rec",
                                             name="rec")
                            nc.vector.reciprocal_approx_fast(out=rec[:],
                                                             in_=ps_den[:])
                            for j, (_vc, oidx) in enumerate(avs):
                                nc.vector.tensor_tensor(
                                    outT[oidx][:, t0:t0 + t_win],
                                    ps_o[j][:], rec[:], ALU.mult)
                            if p_pf is not None and ti == T // t_win - 1:
                                for k, tt in enumerate(
                                        range(t0 // 128, (t0 + t_win) // 128)):
                                    fold_tt(p_pf, p_fin,
                                            tt, "act" if k % 2 else "dve")
                        mstk.close()

            # ---- regular branch: everything pipelined into attention ----
            with tc.If(pid < 4) as cmp:
                # injection schedule: (mi, ti) -> list of (s, thunk-args)
                # just-in-time projection chains, max ~4-5 per window;
                # window (0,0) also builds V two s-blocks ahead of the AV use
                REG_SCHED = {
                    (0, 0): {1: ("k", 0, 1), 3: ("k", 0, 2),
                             7: ("k", 0, 3), 9: ("q", 0, 2), 11: ("q", 0, 3)},
                    (0, 1): {1: ("k", 1, 0), 3: ("q", 1, 0),
                             5: ("k", 1, 1), 7: ("q", 1, 1)},
                    (1, 0): {1: ("k", 1, 2), 3: ("k", 1, 3),
                             5: ("q", 1, 2), 7: ("q", 1, 3)},
                    (1, 1): {1: ("k", 2, 0), 3: ("k", 2, 1),
                             5: ("q", 2, 0)},
                    (2, 0): {1: ("k", 2, 2), 3: ("q", 2, 1), 5: ("k", 2, 3)},
                    (2, 1): {1: ("q", 2, 2)},
                    (2, 2): {1: ("q", 2, 3)},
                }

                def reg_inject(pool, mi, ti, s):
                    if mi == 0 and ti == 0 and s <= 13:
                        v_chain(pool, s + 2, "dve")
                    c = REG_SCHED.get((mi, ti), {}).get(s)
                    if c is not None:
                        qk_chain(pool, c[0], c[1], c[2])

                attention(maps=[([0], [(0, 0)], 1024, 1, 2, 2),
                                ([1], [(1, 1)], 1024, 1, 2, 2),
                                ([2], [(2, 2)], 512, 1, 1, 2)],
                          scale=SCALE_REG, sb_em=None,
                          inject=reg_inject, fold_last=True)

            # ---- cultural branch: dense Q/K rest, V inside first window ----
            with cmp.Else():
                with tc.tile_pool(name="pps2", bufs=4, space="PSUM") as p_pps2:
                    for tq in (2, 3):
                        qk_chain(p_pps2, "q", 0, tq)
                    for tq in (1, 2, 3):
                        qk_chain(p_pps2, "k", 0, tq)
                    for tq in range(4):
                        for f in (1, 2):
                            qk_chain(p_pps2, "q", f, tq)
                            qk_chain(p_pps2, "k", f, tq)

                def cul_inject(pool, mi, ti, s):
                    if ti == 0 and s <= 13:
                        v_chain(pool, s + 2, "act")

                with tc.tile_pool(name="emp", bufs=1) as p_em:
                    sb_em = p_em.tile([128, NT * T], F16, tag="em", name="sb_em")
                    for sc in range(NT):
                        nc.sync.dma_start(out=sb_em[:, sc * T:(sc + 1) * T],
                                          in_=d_em[sc, :, :])
                    attention(maps=[([0, 1, 2], [(0, 0), (1, 1), (2, 2)], 512, 1,
                                     0, 3)],
                              scale=SCALE_CUL, sb_em=sb_em,
                              inject=cul_inject, fold_last=True)
    nc.compile()
    return nc


def _get_nc():
    global _NC_CACHE
    if _NC_CACHE is None:
        _NC_CACHE = _build_nc()
    return _NC_CACHE


def _chunked_T(a, dt=np.float16):
    """[E, X]-style [768, X] -> [128, 6*X] with e-chunk-major free layout."""
    e, x = a.shape
    return np.ascontiguousarray(
        a.reshape(e // 128, 128, x).transpose(1, 0, 2).reshape(128, (e // 128) * x)
    ).astype(dt)


def kernel(hidden_states, cultural_mask, attention_mask,
           rq_w, rk_w, rv_w, ro_w, cq_w, ck_w, cv_w, co_w,
           rq_b, rk_b, rv_b, ro_b, cq_b, ck_b, cv_b, co_b,
           r_cb, c_cb, out_w, out_b):
    hidden_states = np.asarray(hidden_states)
    nc = _get_nc()
    Wo1 = np.asarray(out_w[:E], np.float64)
    Wo2 = np.asarray(out_w[E:], np.float64)
    wfold_reg = (np.asarray(ro_w, np.float64) @ Wo1)
    wfold_cul = (np.asarray(co_w, np.float64) @ Wo2)
    r_cb_flat = np.asarray(r_cb, np.float64).reshape(-1)  # [768]
    c_cb_flat = np.asarray(c_cb, np.float64).reshape(-1)  # [768]
    qb_reg_full = np.asarray(rq_b, np.float64) + r_cb_flat
    qb_cul_full = np.asarray(cq_b, np.float64) + c_cb_flat

    em_zero = np.zeros((NT, 128, T), np.float16)
    attn_np = np.asarray(attention_mask, np.float32)
    in_maps = []
    for core in range(8):
        if core < 4:
            b, h0 = core // 2, (core % 2) * 3
            cols = slice(h0 * 128, h0 * 128 + F)
            wq_l, wk_l, wv_l = rq_w[:, cols], rk_w[:, cols], rv_w[:, cols]
            qb_l = qb_reg_full[cols]
            kb_l = np.asarray(rk_b, np.float64)[cols]
            wfold_l = wfold_reg[cols]
            em_l = em_zero
        else:
            b, h = (core - 4) // 2, (core - 4) % 2
            cols = slice(h * F, (h + 1) * F)
            wq_l, wk_l, wv_l = cq_w[:, cols], ck_w[:, cols], cv_w[:, cols]
            qb_l = qb_cul_full[cols]
            kb_l = np.asarray(ck_b, np.float64)[cols]
            wfold_l = wfold_cul[cols]
            # exp(cultural_mask^T + attention_mask[s]) as [s_chunk, p, t]
            em_f = np.exp(np.asarray(cultural_mask[b], np.float64).T
                          + attn_np[b, 0, 0, :][:, None])
            em_l = np.ascontiguousarray(
                em_f.reshape(NT, 128, T)).astype(np.float16)
        xT = np.asarray(hidden_states[b], np.float32).T  # [768, 2048]
        xT_c = _chunked_T(np.ascontiguousarray(xT))
        im = {
            "wq": _chunked_T(np.asarray(wq_l, np.float32)),
            "wk": _chunked_T(np.asarray(wk_l, np.float32)),
            "wv": _chunked_T(np.asarray(wv_l, np.float32)),
            "qb": np.ascontiguousarray(np.asarray(qb_l, np.float32).reshape(NF, 128).T),
            "kb": np.ascontiguousarray(np.asarray(kb_l, np.float32).reshape(NF, 128).T),
            "attn": np.ascontiguousarray(attn_np[b, 0, 0, :].reshape(NT, 128).T),
            "wfold": _chunked_T(np.asarray(wfold_l, np.float32)),
            "em": em_l,
        }
        for e in range(NE):
            im[f"xT{e}"] = np.ascontiguousarray(xT_c[:, e * T:(e + 1) * T])
        in_maps.append(im)

    res = run_bass_kernel_spmd(nc, in_maps, list(range(8))).results

    bias_total = (np.asarray(out_b, np.float64)
                  + np.asarray(ro_b, np.float64) @ Wo1
                  + np.asarray(co_b, np.float64) @ Wo2
                  + np.asarray(rv_b, np.float64) @ np.asarray(ro_w, np.float64) @ Wo1
                  + np.asarray(cv_b, np.float64) @ np.asarray(co_w, np.float64) @ Wo2)
    out = np.empty((B, T, E), np.float32)
    for b in range(B):
        acc = (res[2 * b]["out"].astype(np.float64)
               + res[2 * b + 1]["out"].astype(np.float64)
               + res[4 + 2 * b]["out"].astype(np.float64)
               + res[5 + 2 * b]["out"].astype(np.float64)
               + bias_total)
        out[b] = acc.astype(np.float32)
    return out


# revision 22
# speedup vs baseline: 1.0278x; 1.0278x over previous
"""Trainium2 Bass kernel for nn_MultiHeadCulturalAttention.

Sharding (8 cores, SPMD single program with a partition-id branch):
  cores 0-3: "regular" branch — (batch b = core//2), 3 heads of hd=128 each
  cores 4-7: "cultural" branch — (batch b = (core-4)//2), 1 head of hd=384

All streaming operands are float16 (PE runs 16-bit at full rate with fast
weight loads; DVE gets 2-4x modes); accumulation is fp32 in PSUM.
Softmax denominators come from DVE-accumulated exp tiles reduced by a
ones[128,128] broadcast-matmul per t-window, then a fast-approx DVE
reciprocal + multiply (no gpsimd, no DMA round-trips).

The regular branch's attention is exp-paced on the scalar engine, so all
remaining PE work is software-pipelined into its idle slots: only Q/K of
head-chunk 0 are projected up front; the V projection runs inside the
first attention window, Q/K of chunks 1-2 inside later windows, and the
folded output projection (wfold = branch_out_w @ out_w_half) inside the
last map. The cultural branch (PE-paced) projects Q/K densely, folds at
the end. Host sums 4 fp32 partials per batch plus a constant bias row.
"""
import numpy as np

import concourse.bass as bass
import concourse.mybir as mybir
from concourse import bacc
from concourse.tile import TileContext
from concourse.bass_utils import run_bass_kernel_spmd

F32 = mybir.dt.float32
F16 = mybir.dt.float16
AF = mybir.ActivationFunctionType
ALU = mybir.AluOpType

B, T, E = 2, 2048, 768
NE = E // 128            # 6 e-chunks
NT = T // 128            # 16 tiles along seq
F = 384                  # per-core projection width (3 reg heads / 1 cul head)
NF = F // 128            # 3 f-tiles
SCALE_REG = float(128 ** -0.5)
SCALE_CUL = float(384 ** -0.5)
GRP = 8                  # exp-tile accumulation group size

_NC_CACHE = None


def _build_nc():
    nc = bacc.Bacc()
    d_xT = [nc.declare_dram_parameter(f"xT{e}", [128, T], F16, isOutput=False)
            for e in range(NE)]
    d_wq = nc.declare_dram_parameter("wq", [128, NE * F], F16, isOutput=False)
    d_wk = nc.declare_dram_parameter("wk", [128, NE * F], F16, isOutput=False)
    d_wv = nc.declare_dram_parameter("wv", [128, NE * F], F16, isOutput=False)
    d_qb = nc.declare_dram_parameter("qb", [128, NF], F32, isOutput=False)
    d_kb = nc.declare_dram_parameter("kb", [128, NF], F32, isOutput=False)
    d_attn = nc.declare_dram_parameter("attn", [128, NT], F32, isOutput=False)
    d_wfold = nc.declare_dram_parameter("wfold", [128, NF * E], F16, isOutput=False)
    d_em = nc.declare_dram_parameter("em", [NT, 128, T], F16, isOutput=False)
    d_out = nc.declare_dram_parameter("out", [T, E], F32, isOutput=True)

    with TileContext(nc) as tc:
        pid = nc.partition_id()
        from contextlib import ExitStack
        with ExitStack() as stk:
            # ---- persistent pools ----
            p_small = stk.enter_context(tc.tile_pool(name="small", bufs=1))
            p_qt = stk.enter_context(tc.tile_pool(name="qt", bufs=1))
            p_kt = stk.enter_context(tc.tile_pool(name="kt", bufs=1))
            p_v = stk.enter_context(tc.tile_pool(name="vp", bufs=1))
            p_outT = stk.enter_context(tc.tile_pool(name="outT", bufs=1))
            p_wfold = stk.enter_context(tc.tile_pool(name="wfp", bufs=1))
            p_xw = stk.enter_context(tc.tile_pool(name="xw", bufs=1))

            sb_ones = p_small.tile([128, 512], F16)
            sb_qb = p_small.tile([128, NF], F32)
            sb_kb = p_small.tile([128, NF], F32)
            sb_attn = p_small.tile([128, NT], F32)
            sb_wfold = p_wfold.tile([128, NF * E], F16)
            nc.vector.memset(sb_ones[:], 1.0)
            nc.sync.dma_start(out=sb_qb[:], in_=d_qb[:])
            nc.sync.dma_start(out=sb_kb[:], in_=d_kb[:])
            nc.sync.dma_start(out=sb_attn[:], in_=d_attn[:])

            outT = [p_outT.tile([128, T], F16, tag=f"outT{j}", name=f"outT{j}")
                    for j in range(NF)]
            sb_q = [p_qt.tile([128, T], F16, tag=f"q{f}", name=f"qf{f}")
                    for f in range(NF)]
            sb_k = [p_kt.tile([128, T], F16, tag=f"k{f}", name=f"kf{f}")
                    for f in range(NF)]
            sb_v = [p_v.tile([128, F], F16, tag=f"v{s}", name=f"vs{s}")
                    for s in range(NT)]

            sb_wq = p_xw.tile([128, NE * F], F16, tag="wq")
            sb_wk = p_xw.tile([128, NE * F], F16, tag="wk")
            sb_wv = p_xw.tile([128, NE * F], F16, tag="wv")
            sb_x = p_xw.tile([128, NE * T], F16, tag="x")
            nc.sync.dma_start(out=sb_wq[:], in_=d_wq[:])
            nc.sync.dma_start(out=sb_wk[:], in_=d_wk[:])
            nc.sync.dma_start(out=sb_wv[:], in_=d_wv[:])
            for tq in range(4):
                for e in range(NE):
                    nc.sync.dma_start(
                        out=sb_x[:, e * T + tq * 512: e * T + (tq + 1) * 512],
                        in_=d_xT[e][:, tq * 512:(tq + 1) * 512])
            nc.sync.dma_start(out=sb_wfold[:], in_=d_wfold[:])

            # ---- projection chain helpers (emit into a given PSUM pool) ----
            def qk_chain(pool, which, f, tq):
                """One [128,512] projection chain for Q^T/K^T chunk f,
                T-quarter tq; DVE evacuation with per-partition bias."""
                sb_w = sb_wq if which == "q" else sb_wk
                sb_b = sb_qb if which == "q" else sb_kb
                dst = sb_q[f] if which == "q" else sb_k[f]
                c0 = tq * 512
                ps = pool.tile([128, 512], F32, tag="psc", name="ps_qk")
                for e in range(NE):
                    nc.tensor.matmul(
                        ps[:],
                        lhsT=sb_w[:, e * F + f * 128: e * F + (f + 1) * 128],
                        rhs=sb_x[:, e * T + c0: e * T + c0 + 512],
                        start=(e == 0), stop=(e == NE - 1))
                nc.vector.tensor_scalar_add(dst[:, c0:c0 + 512], ps[:],
                                            sb_b[:, f:f + 1])

            def v_chain(pool, s, engine="dve"):
                """V[s-block]: [128, F] = sum_e x^T[e, s]ᵀ wv[e]."""
                ps = pool.tile([128, F], F32, tag="psc", name="ps_vc")
                for e in range(NE):
                    nc.tensor.matmul(
                        ps[:], lhsT=sb_x[:, e * T + s * 128:(e * T) + (s + 1) * 128],
                        rhs=sb_wv[:, e * F:(e + 1) * F],
                        start=(e == 0), stop=(e == NE - 1))
                if engine == "dve":
                    nc.vector.tensor_copy(sb_v[s][:], ps[:])
                else:
                    nc.scalar.activation(sb_v[s][:], ps[:], AF.Copy,
                                         bias=0.0, scale=1.0)

            def fold_tt(p_pf, p_fin, tt, fin_engine):
                ps_f = p_pf.tile([128, E], F32, tag="pf", name="ps_f")
                for c in range(NF):
                    for e0, e1 in ((0, 512), (512, 768)):
                        nc.tensor.matmul(
                            ps_f[:, e0:e1],
                            lhsT=outT[c][:, tt * 128:(tt + 1) * 128],
                            rhs=sb_wfold[:, c * E + e0: c * E + e1],
                            start=(c == 0), stop=(c == NF - 1))
                fin = p_fin.tile([128, E], F32, tag="fin", name="fin")
                if fin_engine == "act":
                    nc.scalar.activation(fin[:], ps_f[:], AF.Copy,
                                         bias=0.0, scale=1.0)
                else:
                    nc.vector.tensor_copy(fin[:], ps_f[:])
                nc.sync.dma_start(out=d_out[tt * 128:(tt + 1) * 128, :], in_=fin[:])

            # ---- common: warmup + just enough Q0/K0/V for window (0,0) ----
            with tc.tile_pool(name="wu", bufs=2, space="PSUM") as p_wu:
                ps_wu = p_wu.tile([128, 512], F32, tag="wup", name="ps_wu")
                for _ in range(20):
                    nc.tensor.matmul(ps_wu[:], lhsT=sb_ones[:, 0:128],
                                     rhs=sb_ones[:], start=True, stop=True)
                qk_chain(p_wu, "q", 0, 0)
                qk_chain(p_wu, "k", 0, 0)
                v_chain(p_wu, 0)
                v_chain(p_wu, 1)
                qk_chain(p_wu, "q", 0, 1)

            n_grp = NT // GRP

            def attention(maps, scale, sb_em, inject, fold_last):
                """maps: (chunks, avs, t_win) per map; inject(pool, mi, ti, s)
                issues extra PE work into the exp-paced pipeline. fold_last:
                open fold pools inside the last map; window ti's fold runs
                deferred inside window ti+1's s-loop (the final window's
                fold trails)."""
                from contextlib import ExitStack as ES
                with ES() as astk:
                    p_wt = astk.enter_context(tc.tile_pool(name="wt", bufs=3))
                    p_acc = astk.enter_context(tc.tile_pool(name="acc", bufs=2))
                    p_rec = astk.enter_context(tc.tile_pool(name="rec", bufs=2))
                    for mi, (chunks, avs, t_win, po_bufs, inj_bufs,
                             psc_bufs) in enumerate(maps):
                        n_tq = t_win // 512
                        n_tt = t_win // 128
                        mstk = ES()
                        p_psc = mstk.enter_context(tc.tile_pool(
                            name=f"psc{mi}", bufs=psc_bufs, space="PSUM"))
                        p_po = mstk.enter_context(tc.tile_pool(
                            name=f"po{mi}", bufs=po_bufs, space="PSUM"))
                        p_inj = mstk.enter_context(tc.tile_pool(
                            name=f"pinj{mi}", bufs=inj_bufs, space="PSUM")) \
                            if inj_bufs else None
                        p_pf = p_fin = None
                        if fold_last and mi == len(maps) - 1:
                            pf_bufs = 2 if t_win == 512 and len(maps) > 1 else 1
                            p_pf = mstk.enter_context(tc.tile_pool(
                                name="pfr", bufs=pf_bufs, space="PSUM"))
                            p_fin = mstk.enter_context(
                                tc.tile_pool(name="finr", bufs=3))
                        fold_stride = NT // n_tt
                        for ti in range(T // t_win):
                            t0 = ti * t_win
                            ps_o = [p_po.tile([128, t_win], F32, tag=f"po{j}",
                                              name=f"ps_o{j}")
                                    for j in range(len(avs))]
                            accs = [p_acc.tile([128, t_win], F16, tag=f"acc{g}",
                                               name=f"acc{g}")
                                    for g in range(n_grp)]
                            for s in range(NT):
                                inject(p_inj if p_inj is not None else p_psc,
                                       mi, ti, s)
                                if (p_pf is not None and ti > 0
                                        and s % fold_stride == fold_stride - 1):
                                    tt = (ti - 1) * n_tt + s // fold_stride
                                    fold_tt(p_pf, p_fin, tt, "dve")
                                ps_sc = p_psc.tile([128, t_win], F32, tag="psc",
                                                   name="ps_sc")
                                for ci, c in enumerate(chunks):
                                    for tq in range(n_tq):
                                        nc.tensor.matmul(
                                            ps_sc[:, tq * 512:(tq + 1) * 512],
                                            lhsT=sb_k[c][:, s * 128:(s + 1) * 128],
                                            rhs=sb_q[c][:, t0 + tq * 512:
                                                        t0 + (tq + 1) * 512],
                                            start=(ci == 0),
                                            stop=(ci == len(chunks) - 1))
                                wt = p_wt.tile([128, t_win], F16, tag="wt",
                                               name="wt")
                                if sb_em is not None:
                                    wt0 = p_wt.tile([128, t_win], F16, tag="wt0",
                                                    name="wt0")
                                    nc.scalar.activation(wt0[:], ps_sc[:], AF.Exp,
                                                         bias=0.0, scale=scale)
                                    nc.vector.tensor_tensor(
                                        wt[:], wt0[:],
                                        sb_em[:, s * T + t0: s * T + t0 + t_win],
                                        ALU.mult)
                                else:
                                    nc.scalar.activation(
                                        wt[:], ps_sc[:], AF.Exp,
                                        bias=sb_attn[:, s:s + 1], scale=scale)
                                g = s // GRP
                                if s % GRP == 0:
                                    nc.vector.tensor_copy(accs[g][:], wt[:])
                                else:
                                    nc.vector.tensor_tensor(accs[g][:], accs[g][:],
                                                            wt[:], ALU.add)
                                for j, (vc, _oidx) in enumerate(avs):
                                    for tq in range(n_tq):
                                        nc.tensor.matmul(
                                            ps_o[j][:, tq * 512:(tq + 1) * 512],
                                            lhsT=sb_v[s][:, vc * 128:(vc + 1) * 128],
                                            rhs=wt[:, tq * 512:(tq + 1) * 512],
                                            start=(s == 0), stop=(s == NT - 1))
                            ps_den = p_psc.tile([128, t_win], F32, tag="psc",
                                                name="ps_den")
                            for g in range(n_grp):
                                for tq in range(n_tq):
                                    nc.tensor.matmul(
                                        ps_den[:, tq * 512:(tq + 1) * 512],
                                        lhsT=sb_ones[:, 0:128],
                                        rhs=accs[g][:, tq * 512:(tq + 1) * 512],
                                        start=(g == 0), stop=(g == n_grp - 1))
                            rec = p_rec.tile([128, t_win], F32, tag="rec",
                                             name="rec")
                            nc.vector.reciprocal_approx_fast(out=rec[:],
                                                             in_=ps_den[:])
                            for j, (_vc, oidx) in enumerate(avs):
                                nc.vector.tensor_tensor(
                                    outT[oidx][:, t0:t0 + t_win],
                                    ps_o[j][:], rec[:], ALU.mult)
                            if p_pf is not None and ti == T // t_win - 1:
                                for k, tt in enumerate(
                                        range(t0 // 128, (t0 + t_win) // 128)):
                                    fold_tt(p_pf, p_fin,
                                            tt, "act" if k % 2 else "dve")
                        mstk.close()

            # ---- regular branch: everything pipelined into attention ----
            with tc.If(pid < 4) as cmp:
                # injection schedule: (mi, ti) -> list of (s, thunk-args)
                # just-in-time projection chains, max ~4-5 per window;
                # window (0,0) also builds V two s-blocks ahead of the AV use
                REG_SCHED = {
                    (0, 0): {1: ("k", 0, 1), 3: ("k", 0, 2),
                             7: ("k", 0, 3), 9: ("q", 0, 2), 11: ("q", 0, 3)},
                    (0, 1): {1: ("k", 1, 0), 3: ("q", 1, 0),
                             5: ("k", 1, 1), 7: ("q", 1, 1)},
                    (1, 0): {1: ("k", 1, 2), 3: ("k", 1, 3),
                             5: ("q", 1, 2), 7: ("q", 1, 3)},
                    (1, 1): {1: ("k", 2, 0), 3: ("k", 2, 1),
                             5: ("q", 2, 0)},
                    (2, 0): {1: ("k", 2, 2), 3: ("q", 2, 1), 5: ("k", 2, 3)},
                    (2, 1): {1: ("q", 2, 2)},
                    (2, 2): {1: ("q", 2, 3)},
                }

                def reg_inject(pool, mi, ti, s):
                    if mi == 0 and ti == 0 and s <= 13:
                        v_chain(pool, s + 2, "dve")
                    c = REG_SCHED.get((mi, ti), {}).get(s)
                    if c is not None:
                        qk_chain(pool, c[0], c[1], c[2])

                attention(maps=[([0], [(0, 0)], 1024, 1, 2, 2),
                                ([1], [(1, 1)], 1024, 1, 2, 2),
                                ([2], [(2, 2)], 512, 1, 1, 2)],
                          scale=SCALE_REG, sb_em=None,
                          inject=reg_inject, fold_last=True)

            # ---- cultural branch: dense Q/K rest, V inside first window ----
            with cmp.Else():
                with tc.tile_pool(name="pps2", bufs=4, space="PSUM") as p_pps2:
                    for tq in (2, 3):
                        qk_chain(p_pps2, "q", 0, tq)
                    for tq in (1, 2, 3):
                        qk_chain(p_pps2, "k", 0, tq)
                    for tq in range(4):
                        for f in (1, 2):
                            qk_chain(p_pps2, "q", f, tq)
                            qk_chain(p_pps2, "k", f, tq)

                def cul_inject(pool, mi, ti, s):
                    if ti == 0 and s <= 13:
                        v_chain(pool, s + 2, "act")

                with tc.tile_pool(name="emp", bufs=1) as p_em:
                    sb_em = p_em.tile([128, NT * T], F16, tag="em", name="sb_em")
                    for sc in range(NT):
                        nc.sync.dma_start(out=sb_em[:, sc * T:(sc + 1) * T],
                                          in_=d_em[sc, :, :])
                    attention(maps=[([0, 1, 2], [(0, 0), (1, 1), (2, 2)], 512, 1,
                                     0, 3)],
                              scale=SCALE_CUL, sb_em=sb_em,
                              inject=cul_inject, fold_last=True)
    nc.compile()
    return nc


def _get_nc():
    global _NC_CACHE
    if _NC_CACHE is None:
        _NC_CACHE = _build_nc()
    return _NC_CACHE


def _chunked_T(a, dt=np.float16):
    """[E, X]-style [768, X] -> [128, 6*X] with e-chunk-major free layout."""
    e, x = a.shape
    return np.ascontiguousarray(
        a.reshape(e // 128, 128, x).transpose(1, 0, 2).reshape(128, (e // 128) * x)
    ).astype(dt)


def kernel(hidden_states, cultural_mask, attention_mask,
           rq_w, rk_w, rv_w, ro_w, cq_w, ck_w, cv_w, co_w,
           rq_b, rk_b, rv_b, ro_b, cq_b, ck_b, cv_b, co_b,
           r_cb, c_cb, out_w, out_b):
    hidden_states = np.asarray(hidden_states)
    nc = _get_nc()
    Wo1 = np.asarray(out_w[:E], np.float64)
    Wo2 = np.asarray(out_w[E:], np.float64)
    wfold_reg = (np.asarray(ro_w, np.float64) @ Wo1)
    wfold_cul = (np.asarray(co_w, np.float64) @ Wo2)
    r_cb_flat = np.asarray(r_cb, np.float64).reshape(-1)  # [768]
    c_cb_flat = np.asarray(c_cb, np.float64).reshape(-1)  # [768]
    qb_reg_full = np.asarray(rq_b, np.float64) + r_cb_flat
    qb_cul_full = np.asarray(cq_b, np.float64) + c_cb_flat

    em_zero = np.zeros((NT, 128, T), np.float16)
    attn_np = np.asarray(attention_mask, np.float32)
    in_maps = []
    for core in range(8):
        if core < 4:
            b, h0 = core // 2, (core % 2) * 3
            cols = slice(h0 * 128, h0 * 128 + F)
            wq_l, wk_l, wv_l = rq_w[:, cols], rk_w[:, cols], rv_w[:, cols]
            qb_l = qb_reg_full[cols]
            kb_l = np.asarray(rk_b, np.float64)[cols]
            wfold_l = wfold_reg[cols]
            em_l = em_zero
        else:
            b, h = (core - 4) // 2, (core - 4) % 2
            cols = slice(h * F, (h + 1) * F)
            wq_l, wk_l, wv_l = cq_w[:, cols], ck_w[:, cols], cv_w[:, cols]
            qb_l = qb_cul_full[cols]
            kb_l = np.asarray(ck_b, np.float64)[cols]
            wfold_l = wfold_cul[cols]
            # exp(cultural_mask^T + attention_mask[s]) as [s_chunk, p, t]
            em_f = np.exp(np.asarray(cultural_mask[b], np.float64).T
                          + attn_np[b, 0, 0, :][:, None])
            em_l = np.ascontiguousarray(
                em_f.reshape(NT, 128, T)).astype(np.float16)
        xT = np.asarray(hidden_states[b], np.float32).T  # [768, 2048]
        xT_c = _chunked_T(np.ascontiguousarray(xT))
        im = {
            "wq": _chunked_T(np.asarray(wq_l, np.float32)),
            "wk": _chunked_T(np.asarray(wk_l, np.float32)),
            "wv": _chunked_T(np.asarray(wv_l, np.float32)),
            "qb": np.ascontiguousarray(np.asarray(qb_l, np.float32).reshape(NF, 128).T),
            "kb": np.ascontiguousarray(np.asarray(kb_l, np.float32).reshape(NF, 128).T),
            "attn": np.ascontiguousarray(attn_np[b, 0, 0, :].reshape(NT, 128).T),
            "wfold": _chunked_T(np.asarray(wfold_l, np.float32)),
            "em": em_l,
        }
        for e in range(NE):
            im[f"xT{e}"] = np.ascontiguousarray(xT_c[:, e * T:(e + 1) * T])
        in_maps.append(im)

    res = run_bass_kernel_spmd(nc, in_maps, list(range(8))).results

    bias_total = (np.asarray(out_b, np.float64)
                  + np.asarray(ro_b, np.float64) @ Wo1
                  + np.asarray(co_b, np.float64) @ Wo2
                  + np.asarray(rv_b, np.float64) @ np.asarray(ro_w, np.float64) @ Wo1
                  + np.asarray(cv_b, np.float64) @ np.asarray(co_w, np.float64) @ Wo2)
    out = np.empty((B, T, E), np.float32)
    for b in range(B):
        acc = (res[2 * b]["out"].astype(np.float64)
               + res[2 * b + 1]["out"].astype(np.float64)
               + res[4 + 2 * b]["out"].astype(np.float64)
               + res[5 + 2 * b]["out"].astype(np.float64)
               + bias_total)
        out[b] = acc.astype(np.float32)
    return out
